# revision 7
# baseline (speedup 1.0000x reference)
"""VQ codebook forward (nn_CodeBook) on 8 Trainium2 NeuronCores.

Math (per row i of c_input):
    dist[i,k] = |x_i|^2 + |e_k|^2 - 2 x_i.e_k
    argmin_k dist = argmax_k negdist,  negdist[i,k] = 2 x_i.e_k - |e_k|^2
    min-dist     = |x_i|^2 - m_i,      m_i = max_k negdist[i,k]
    loss = 1.25 * mean(min-dist)   (q/e latent losses are numerically equal)
    quantized_st == e[argmin]      (straight-through is identity in value)
    encodings = one_hot(argmin)

Device kernel per 128-row tile (rows on partitions, K=1024 on free dim):
    PE   : negdist2 = 2 x.e^T via matmul (x pre-transposed on host)
    DVE  : tensor_tensor_reduce fuses PSUM evacuation, -|e|^2 bias and
           row-max m in one op
    ACT  : one-hot = Exp(2^50*(negdist - m)) -- exact {0.0, 1.0} since the
           pow-2 scale keeps the cancellation sign-exact; Square+accum for
           |x|^2 partial sums
    DVE/GpSimd/ACT: idx = sum_k onehot*k split across engines
    SWDGE: q rows gathered from the codebook with per-partition indirect DMA
Loss is assembled on host from per-core partial sums of m and |x|^2.
"""
import sys

sys.path.insert(0, "/opt/trn_rl_repo")

import numpy as np

N, K, D = 131072, 1024, 128
NCORES = 8
NSHARD = N // NCORES          # 16384 rows per core
NT = NSHARD // 128            # 128 tiles per core
SCALE = float(2.0 ** 50)
SPLIT = 524                   # idx columns handled by DVE stt; rest GpSimd+ACT

_CACHE = {}
NO_Q = False     # debug: skip indirect q gather
NO_ENC = False   # debug: skip enc DMA out


def _build_nc(n_tiles):
    import concourse.bacc as bacc
    import concourse.mybir as mybir
    import concourse.tile as tile
    from concourse import bass

    ns = n_tiles * 128
    nc = bacc.Bacc("TRN2", target_bir_lowering=False, debug=False, num_devices=NCORES)

    xt = nc.declare_dram_parameter("xt", [D, ns], mybir.dt.float32, isOutput=False)
    # blob: [:, 0:K] = 2*e.T ; [:, K:2K] = iota ;
    # row0 [2K:3K] = -|e|^2 (seed row) ; row0 [3K:3K+128] = 1.0 (ones row)
    blob = nc.declare_dram_parameter("blob", [128, 3 * K + 128], mybir.dt.float32, isOutput=False)
    e_nat = nc.declare_dram_parameter("e_nat", [K, D], mybir.dt.float32, isOutput=False)

    enc_out = nc.declare_dram_parameter("enc_out", [ns, K], mybir.dt.float32, isOutput=True)
    q_out = nc.declare_dram_parameter("q_out", [ns, D], mybir.dt.float32, isOutput=True)
    stats = nc.declare_dram_parameter("stats", [128, 2], mybir.dt.float32, isOutput=True)

    f32 = mybir.dt.float32
    Alu = mybir.AluOpType
    Act = mybir.ActivationFunctionType

    with tile.TileContext(nc) as tc:
        with (
            tc.tile_pool(name="const", bufs=1) as const,
            tc.tile_pool(name="acc", bufs=1) as acc,
            tc.tile_pool(name="xin", bufs=3) as xin,
            tc.tile_pool(name="nd", bufs=3) as ndp,
            tc.tile_pool(name="enc", bufs=3) as encp,
            tc.tile_pool(name="junk", bufs=2) as junkp,
            tc.tile_pool(name="small", bufs=4) as smallp,
            tc.tile_pool(name="q", bufs=3) as qp,
            tc.tile_pool(name="ps", bufs=3, space="PSUM") as psp,
        ):
            blob_t = const.tile([128, 3 * K + 128], f32)
            nc.sync.dma_start(blob_t[:], blob[:])
            et2 = blob_t[:, 0:K]
            iota = blob_t[:, K:2 * K]
            seedneg = blob_t[0:1, 2 * K:3 * K]
            ones_row = blob_t[0:1, 3 * K:3 * K + 128]

            m_all = acc.tile([128, n_tiles], f32)
            xsq_all = acc.tile([128, n_tiles], f32)

            for t in range(n_tiles):
                xt_t = xin.tile([D, 128], f32)
                nc.sync.dma_start(xt_t[:], xt[:, t * 128:(t + 1) * 128])

                nd_ps = psp.tile([128, K], f32)
                for h in range(2):
                    sl = slice(h * 512, (h + 1) * 512)
                    nc.tensor.matmul(nd_ps[:, sl], ones_row, seedneg[:, sl], start=True, stop=False)
                    nc.tensor.matmul(nd_ps[:, sl], xt_t[:], et2[:, sl], start=False, stop=True)

                nc.vector.tensor_reduce(m_all[:, t:t + 1], nd_ps[:],
                                        axis=mybir.AxisListType.X, op=Alu.max)

                bias_t = smallp.tile([128, 1], f32)
                nc.vector.tensor_scalar_mul(bias_t[:], m_all[:, t:t + 1], -SCALE)

                enc_t = encp.tile([128, K], f32)
                nc.scalar.activation(enc_t[:], nd_ps[:], Act.Exp, bias=bias_t[:], scale=SCALE)

                # idx = sum_k enc*k, split DVE | GpSimd+ACT
                junk_a = junkp.tile([128, SPLIT], f32, tag="ja")
                idx_a = smallp.tile([128, 1], f32, tag="ia")
                nc.vector.scalar_tensor_tensor(
                    junk_a[:], enc_t[:, :SPLIT], 0.0, iota[:, :SPLIT],
                    op0=Alu.add, op1=Alu.mult, accum_out=idx_a[:])
                junk_b = junkp.tile([128, K - SPLIT], f32, tag="jb")
                nc.gpsimd.tensor_tensor(junk_b[:], enc_t[:, SPLIT:], iota[:, SPLIT:], op=Alu.mult)
                junk_b2 = junkp.tile([128, K - SPLIT], f32, tag="jb2")
                idx_b = smallp.tile([128, 1], f32, tag="ib")
                nc.scalar.activation(junk_b2[:], junk_b[:], Act.Copy, accum_out=idx_b[:])
                idx_f = smallp.tile([128, 1], f32, tag="if")
                nc.vector.tensor_add(idx_f[:], idx_a[:], idx_b[:])

                junk_x = junkp.tile([D, 128], f32, tag="jx")
                nc.scalar.activation(junk_x[:], xt_t[:], Act.Square,
                                     accum_out=xsq_all[:, t:t + 1])

                if not NO_ENC:
                    nc.sync.dma_start(enc_out[t * 128:(t + 1) * 128, :], enc_t[:])

                idx_i = smallp.tile([128, 1], mybir.dt.int32, tag="ii")
                nc.vector.tensor_copy(idx_i[:], idx_f[:])
                q_t = qp.tile([128, D], f32)
                if NO_Q:
                    nc.vector.memset(q_t[:], 0.0)
                else:
                    nc.gpsimd.indirect_dma_start(
                        out=q_t[:], out_offset=None, in_=e_nat[:],
                        in_offset=bass.IndirectOffsetOnAxis(ap=idx_i[:, :1], axis=0))
                nc.sync.dma_start(q_out[t * 128:(t + 1) * 128, :], q_t[:])

            st = acc.tile([128, 2], f32)
            nc.vector.tensor_reduce(st[:, 0:1], m_all[:], axis=mybir.AxisListType.X, op=Alu.add)
            nc.vector.tensor_reduce(st[:, 1:2], xsq_all[:], axis=mybir.AxisListType.X, op=Alu.add)
            nc.sync.dma_start(stats[:], st[:])

    nc.compile()
    return nc


def _get_nc(n_tiles=NT):
    if n_tiles not in _CACHE:
        _CACHE[n_tiles] = _build_nc(n_tiles)
    return _CACHE[n_tiles]


def _host_inputs(c_input, embedding_weight):
    e = np.ascontiguousarray(embedding_weight, dtype=np.float32)
    et2 = (2.0 * e.T).astype(np.float32)
    esq = np.sum(e.astype(np.float32) ** 2, axis=1).astype(np.float32)
    blob = np.zeros((128, 3 * K + 128), np.float32)
    blob[:, :K] = et2
    blob[:, K:2 * K] = np.arange(K, dtype=np.float32)[None, :]
    blob[0, 2 * K:3 * K] = -esq
    blob[0, 3 * K:3 * K + 128] = 1.0

    in_maps = []
    for c in range(NCORES):
        shard = c_input[c * NSHARD:(c + 1) * NSHARD]
        xt = np.ascontiguousarray(shard.T.astype(np.float32, copy=False))
        in_maps.append({"xt": xt, "blob": blob, "e_nat": e})
    return in_maps


def _run(c_input, embedding_weight, **kw):
    from concourse.bass_utils import run_bass_kernel_spmd

    nc = _get_nc()
    in_maps = _host_inputs(np.asarray(c_input), np.asarray(embedding_weight))
    return run_bass_kernel_spmd(nc, in_maps, list(range(NCORES)), **kw)


def _assemble(results):
    enc = np.concatenate([r["enc_out"] for r in results], axis=0)
    q = np.concatenate([r["q_out"] for r in results], axis=0)
    sse = 0.0
    for r in results:
        st = r["stats"].astype(np.float64)
        sse += st[:, 1].sum() - st[:, 0].sum()
    loss = np.float32(1.25 * sse / (N * D))
    return loss, q, enc


def kernel(c_input, embedding_weight):
    res = _run(c_input, embedding_weight)
    return _assemble(res.results)


def kernel_profiled(c_input, embedding_weight):
    """Returns ((loss, q, enc), BassKernelResults) with trace enabled."""
    res = _run(c_input, embedding_weight, trace=True)
    return _assemble(res.results), res


# revision 11
# speedup vs baseline: 1.6342x; 1.6342x over previous
"""VQ codebook forward (nn_CodeBook) on 8 Trainium2 NeuronCores.

Math (per row i of c_input):
    dist[i,k] = |x_i|^2 + |e_k|^2 - 2 x_i.e_k
    argmin_k dist = argmax_k negdist,  negdist[i,k] = 2 x_i.e_k - |e_k|^2
    min-dist     = |x_i|^2 - m_i,      m_i = max_k negdist[i,k]
    loss = 1.25 * mean(min-dist)   (q/e latent losses are numerically equal)
    quantized_st == e[argmin]      (straight-through is identity in value)
    encodings = one_hot(argmin)

Device kernel per 128-row tile (rows on partitions, K=1024 on free dim):
    PE   : negdist2 = 2 x.e^T via matmul (x pre-transposed on host)
    DVE  : tensor_tensor_reduce fuses PSUM evacuation, -|e|^2 bias and
           row-max m in one op
    ACT  : one-hot = Exp(2^50*(negdist - m)) -- exact {0.0, 1.0} since the
           pow-2 scale keeps the cancellation sign-exact; Square+accum for
           |x|^2 partial sums
    DVE/GpSimd/ACT: idx = sum_k onehot*k split across engines
    SWDGE: q rows gathered from the codebook with per-partition indirect DMA
Loss is assembled on host from per-core partial sums of m and |x|^2.
"""
import sys

sys.path.insert(0, "/opt/trn_rl_repo")

import numpy as np

N, K, D = 131072, 1024, 128
NCORES = 8
NSHARD = N // NCORES          # 16384 rows per core
NT = NSHARD // 128            # 128 tiles per core
SCALE = float(2.0 ** 50)
SPLIT = 524                   # idx columns handled by DVE stt; rest GpSimd+ACT
QG = 1                        # tiles per batched q-gather ([128,G>1] offset APs misbehave on HW)

_CACHE = {}
NO_Q = False     # debug: skip indirect q gather
NO_ENC = False   # debug: skip enc DMA out


def _build_nc(n_tiles):
    import concourse.bacc as bacc
    import concourse.mybir as mybir
    import concourse.tile as tile
    from concourse import bass

    ns = n_tiles * 128
    nc = bacc.Bacc("TRN2", target_bir_lowering=False, debug=False, num_devices=NCORES)

    # x pre-scaled by 2 and split hi/lo in bf16 on host
    xt_hi = nc.declare_dram_parameter("xt_hi", [D, ns], mybir.dt.bfloat16, isOutput=False)
    xt_lo = nc.declare_dram_parameter("xt_lo", [D, ns], mybir.dt.bfloat16, isOutput=False)
    # bf16 blob: [:,0:K]=e.T hi ; [:,K:2K]=e.T lo ;
    # rows0/1 [2K:3K] = -|e|^2 hi / lo ; rows0/1 [3K:3K+128] = 1.0
    blob = nc.declare_dram_parameter("blob", [128, 3 * K + 128], mybir.dt.bfloat16, isOutput=False)
    iotaf = nc.declare_dram_parameter("iotaf", [128, K], mybir.dt.float32, isOutput=False)
    e_nat = nc.declare_dram_parameter("e_nat", [K, D], mybir.dt.float32, isOutput=False)

    enc_out = nc.declare_dram_parameter("enc_out", [ns, K], mybir.dt.float32, isOutput=True)
    q_out = nc.declare_dram_parameter("q_out", [ns, D], mybir.dt.float32, isOutput=True)
    stats = nc.declare_dram_parameter("stats", [128, 1], mybir.dt.float32, isOutput=True)

    f32 = mybir.dt.float32
    Alu = mybir.AluOpType
    Act = mybir.ActivationFunctionType

    with tile.TileContext(nc) as tc:
        with (
            tc.tile_pool(name="const", bufs=1) as const,
            tc.tile_pool(name="acc", bufs=1) as acc,
            tc.tile_pool(name="xin", bufs=3) as xin,
            tc.tile_pool(name="nd", bufs=3) as ndp,
            tc.tile_pool(name="enc", bufs=3) as encp,
            tc.tile_pool(name="junk", bufs=2) as junkp,
            tc.tile_pool(name="small", bufs=4) as smallp,
            tc.tile_pool(name="q", bufs=3) as qp,
            tc.tile_pool(name="ps", bufs=3, space="PSUM") as psp,
        ):
            bf16 = mybir.dt.bfloat16
            blob_t = const.tile([128, 3 * K + 128], bf16)
            nc.sync.dma_start(blob_t[:], blob[:])
            iota_t = const.tile([128, K], f32)
            nc.sync.dma_start(iota_t[:], iotaf[:])
            et_hi = blob_t[:, 0:K]
            et_lo = blob_t[:, K:2 * K]
            seed2 = blob_t[0:2, 2 * K:3 * K]
            ones2 = blob_t[0:2, 3 * K:3 * K + 128]
            iota = iota_t[:, :]

            m_all = acc.tile([128, n_tiles], f32)
            idx_all = acc.tile([128, n_tiles], f32)

            for t in range(n_tiles):
                xh_t = xin.tile([D, 128], bf16, tag="xh")
                nc.sync.dma_start(xh_t[:], xt_hi[:, t * 128:(t + 1) * 128])
                xl_t = xin.tile([D, 128], bf16, tag="xl")
                nc.sync.dma_start(xl_t[:], xt_lo[:, t * 128:(t + 1) * 128])

                nd_ps = psp.tile([128, K], f32)
                for h in range(2):
                    sl = slice(h * 512, (h + 1) * 512)
                    nc.tensor.matmul(nd_ps[:, sl], ones2, seed2[:, sl], start=True, stop=False)
                    nc.tensor.matmul(nd_ps[:, sl], xh_t[:], et_hi[:, sl], start=False, stop=False)
                    nc.tensor.matmul(nd_ps[:, sl], xh_t[:], et_lo[:, sl], start=False, stop=False)
                    nc.tensor.matmul(nd_ps[:, sl], xl_t[:], et_hi[:, sl], start=False, stop=True)

                nc.vector.tensor_reduce(m_all[:, t:t + 1], nd_ps[:],
                                        axis=mybir.AxisListType.X, op=Alu.max)

                bias_t = smallp.tile([128, 1], f32)
                nc.vector.tensor_scalar_mul(bias_t[:], m_all[:, t:t + 1], -SCALE)

                enc_t = encp.tile([128, K], f32)
                nc.scalar.activation(enc_t[:], nd_ps[:], Act.Exp, bias=bias_t[:], scale=SCALE)

                # idx = sum_k enc*k, split DVE | GpSimd+ACT
                junk_a = junkp.tile([128, SPLIT], f32, tag="ja")
                idx_a = smallp.tile([128, 1], f32, tag="ia")
                nc.vector.scalar_tensor_tensor(
                    junk_a[:], enc_t[:, :SPLIT], 0.0, iota[:, :SPLIT],
                    op0=Alu.add, op1=Alu.mult, accum_out=idx_a[:])
                junk_b = junkp.tile([128, K - SPLIT], f32, tag="jb")
                nc.gpsimd.tensor_tensor(junk_b[:], enc_t[:, SPLIT:], iota[:, SPLIT:], op=Alu.mult)
                junk_b2 = junkp.tile([128, K - SPLIT], f32, tag="jb2")
                idx_b = smallp.tile([128, 1], f32, tag="ib")
                nc.scalar.activation(junk_b2[:], junk_b[:], Act.Copy, accum_out=idx_b[:])
                nc.vector.tensor_add(idx_all[:, t:t + 1], idx_a[:], idx_b[:])

                if not NO_ENC:
                    nc.sync.dma_start(enc_out[t * 128:(t + 1) * 128, :], enc_t[:])

                idx_i = smallp.tile([128, 1], mybir.dt.int32, tag="ii")
                nc.vector.tensor_copy(idx_i[:], idx_all[:, t:t + 1])
                q_t = qp.tile([128, D], f32)
                if NO_Q:
                    nc.vector.memset(q_t[:], 0.0)
                else:
                    nc.gpsimd.indirect_dma_start(
                        out=q_t[:], out_offset=None, in_=e_nat[:],
                        in_offset=bass.IndirectOffsetOnAxis(ap=idx_i[:, :1], axis=0))
                nc.sync.dma_start(q_out[t * 128:(t + 1) * 128, :], q_t[:])

            st = acc.tile([128, 1], f32)
            nc.vector.tensor_reduce(st[:, 0:1], m_all[:], axis=mybir.AxisListType.X, op=Alu.add)
            nc.sync.dma_start(stats[:], st[:])

    nc.compile()
    return nc


def _get_nc(n_tiles=NT):
    if n_tiles not in _CACHE:
        _CACHE[n_tiles] = _build_nc(n_tiles)
    return _CACHE[n_tiles]


def _split_bf16(a):
    import ml_dtypes
    hi = a.astype(ml_dtypes.bfloat16)
    lo = (a - hi.astype(np.float32)).astype(ml_dtypes.bfloat16)
    return hi, lo


def _host_inputs(c_input, embedding_weight):
    import ml_dtypes
    e = np.ascontiguousarray(embedding_weight, dtype=np.float32)
    et = e.T.astype(np.float32)
    eh, el = _split_bf16(et)
    esq = np.sum(e ** 2, axis=1).astype(np.float32)
    sh, sl = _split_bf16(-esq)
    blob = np.zeros((128, 3 * K + 128), ml_dtypes.bfloat16)
    blob[:, :K] = eh
    blob[:, K:2 * K] = el
    blob[0, 2 * K:3 * K] = sh
    blob[1, 2 * K:3 * K] = sl
    blob[0:2, 3 * K:3 * K + 128] = 1.0
    iotaf = np.tile(np.arange(K, dtype=np.float32)[None, :], (128, 1))

    in_maps = []
    for c in range(NCORES):
        shard = c_input[c * NSHARD:(c + 1) * NSHARD].astype(np.float32, copy=False)
        x2t = np.ascontiguousarray(2.0 * shard.T)
        xh, xl = _split_bf16(x2t)
        in_maps.append({"xt_hi": np.ascontiguousarray(xh), "xt_lo": np.ascontiguousarray(xl),
                        "blob": blob, "iotaf": iotaf, "e_nat": e})
    return in_maps


def _run(c_input, embedding_weight, **kw):
    from concourse.bass_utils import run_bass_kernel_spmd

    nc = _get_nc()
    in_maps = _host_inputs(np.asarray(c_input), np.asarray(embedding_weight))
    return run_bass_kernel_spmd(nc, in_maps, list(range(NCORES)), **kw)


def _assemble(results, xsq_total):
    enc = np.concatenate([r["enc_out"] for r in results], axis=0)
    q = np.concatenate([r["q_out"] for r in results], axis=0)
    sum_m = sum(r["stats"].astype(np.float64).sum() for r in results)
    loss = np.float32(1.25 * (xsq_total - sum_m) / (N * D))
    return loss, q, enc


def _xsq_total(c_input):
    x = np.asarray(c_input, dtype=np.float32)
    return float(np.sum(x.astype(np.float64) ** 2))


def kernel(c_input, embedding_weight):
    res = _run(c_input, embedding_weight)
    return _assemble(res.results, _xsq_total(c_input))


def kernel_profiled(c_input, embedding_weight):
    """Returns ((loss, q, enc), BassKernelResults) with trace enabled."""
    res = _run(c_input, embedding_weight, trace=True)
    return _assemble(res.results, _xsq_total(c_input)), res


# revision 17
# speedup vs baseline: 1.7113x; 1.0472x over previous
"""VQ codebook forward (nn_CodeBook) on 8 Trainium2 NeuronCores.

Math (per row i of c_input):
    dist[i,k] = |x_i|^2 + |e_k|^2 - 2 x_i.e_k
    argmin_k dist = argmax_k negdist,  negdist[i,k] = 2 x_i.e_k - |e_k|^2
    min-dist     = |x_i|^2 - m_i,      m_i = max_k negdist[i,k]
    loss = 1.25 * mean(min-dist)   (q/e latent losses are numerically equal)
    quantized_st == e[argmin]      (straight-through is identity in value)
    encodings = one_hot(argmin)

Device kernel per 128-row tile (rows on partitions, K=1024 on free dim):
    PE   : negdist2 = 2 x.e^T via matmul (x pre-transposed on host)
    DVE  : tensor_tensor_reduce fuses PSUM evacuation, -|e|^2 bias and
           row-max m in one op
    ACT  : one-hot = Exp(2^50*(negdist - m)) -- exact {0.0, 1.0} since the
           pow-2 scale keeps the cancellation sign-exact; Square+accum for
           |x|^2 partial sums
    DVE/GpSimd/ACT: idx = sum_k onehot*k split across engines
    SWDGE: q rows gathered from the codebook with per-partition indirect DMA
Loss is assembled on host from per-core partial sums of m and |x|^2.
"""
import sys

sys.path.insert(0, "/opt/trn_rl_repo")

import numpy as np

N, K, D = 131072, 1024, 128
NCORES = 8
NSHARD = N // NCORES          # 16384 rows per core
NT = NSHARD // 128            # 128 tiles per core
SCALE = float(2.0 ** 50)
SPLIT = 524                   # idx columns handled by DVE stt; rest GpSimd+ACT
QG = 1                        # per-tile q-gather: HW indirect DMA takes one offset per partition

_CACHE = {}
NO_Q = False     # debug: skip indirect q gather
NO_ENC = False   # debug: skip enc DMA out


def _build_nc(n_tiles):
    import concourse.bacc as bacc
    import concourse.mybir as mybir
    import concourse.tile as tile
    from concourse import bass

    ns = n_tiles * 128
    nc = bacc.Bacc("TRN2", target_bir_lowering=False, debug=False, num_devices=NCORES)

    # x pre-scaled by 2, split hi/lo in bf16 on host, interleaved per 128-col tile:
    # [:, 256t:256t+128] = hi tile t, [:, 256t+128:256t+256] = lo tile t
    xt_hl = nc.declare_dram_parameter("xt_hl", [D, 2 * ns], mybir.dt.bfloat16, isOutput=False)
    # bf16 blob: [:,0:K]=e.T hi ; [:,K:2K]=e.T lo ;
    # rows0/1 [2K:3K] = -|e|^2 hi / lo ; rows0/1 [3K:3K+128] = 1.0
    blob = nc.declare_dram_parameter("blob", [128, 3 * K + 128], mybir.dt.bfloat16, isOutput=False)
    iotaf = nc.declare_dram_parameter("iotaf", [128, K], mybir.dt.float32, isOutput=False)
    e_nat = nc.declare_dram_parameter("e_nat", [K, D], mybir.dt.float32, isOutput=False)

    enc_out = nc.declare_dram_parameter("enc_out", [ns, K], mybir.dt.float32, isOutput=True)
    q_out = nc.declare_dram_parameter("q_out", [ns, D], mybir.dt.float32, isOutput=True)
    stats = nc.declare_dram_parameter("stats", [128, 1], mybir.dt.float32, isOutput=True)

    f32 = mybir.dt.float32
    Alu = mybir.AluOpType
    Act = mybir.ActivationFunctionType

    with tile.TileContext(nc) as tc:
        with (
            tc.tile_pool(name="const", bufs=1) as const,
            tc.tile_pool(name="acc", bufs=1) as acc,
            tc.tile_pool(name="xin", bufs=3) as xin,
            tc.tile_pool(name="nd", bufs=3) as ndp,
            tc.tile_pool(name="enc", bufs=3) as encp,
            tc.tile_pool(name="junk", bufs=2) as junkp,
            tc.tile_pool(name="small", bufs=4) as smallp,
            tc.tile_pool(name="q", bufs=3) as qp,
            tc.tile_pool(name="qidx", bufs=3) as qidxp,
            tc.tile_pool(name="ps", bufs=4, space="PSUM") as psp,
        ):
            bf16 = mybir.dt.bfloat16
            blob_t = const.tile([128, 3 * K + 128], bf16)
            nc.sync.dma_start(blob_t[:], blob[:])
            iota_t = const.tile([128, K], f32)
            nc.sync.dma_start(iota_t[:], iotaf[:])
            et_hi = blob_t[:, 0:K]
            et_lo = blob_t[:, K:2 * K]
            seed2 = blob_t[0:2, 2 * K:3 * K]
            ones2 = blob_t[0:2, 3 * K:3 * K + 128]
            iota = iota_t[:, :]

            m_all = acc.tile([128, n_tiles], f32)

            idx_q = None
            for t in range(n_tiles):
                xhl_t = xin.tile([D, 256], bf16)
                nc.sync.dma_start(xhl_t[:], xt_hl[:, t * 256:(t + 1) * 256])
                xh_t = xhl_t[:, 0:128]
                xl_t = xhl_t[:, 128:256]

                nd_ps = psp.tile([128, K], f32)
                for h in range(2):
                    sl = slice(h * 512, (h + 1) * 512)
                    nc.tensor.matmul(nd_ps[:, sl], ones2, seed2[:, sl], start=True, stop=False)
                    nc.tensor.matmul(nd_ps[:, sl], xh_t, et_hi[:, sl], start=False, stop=False)
                    nc.tensor.matmul(nd_ps[:, sl], xh_t, et_lo[:, sl], start=False, stop=False)
                    nc.tensor.matmul(nd_ps[:, sl], xl_t, et_hi[:, sl], start=False, stop=True)

                nc.vector.tensor_reduce(m_all[:, t:t + 1], nd_ps[:],
                                        axis=mybir.AxisListType.X, op=Alu.max)

                bias_t = smallp.tile([128, 1], f32)
                nc.scalar.mul(bias_t[:], m_all[:, t:t + 1], -SCALE)

                enc_t = encp.tile([128, K], f32)
                nc.scalar.activation(enc_t[:], nd_ps[:], Act.Exp, bias=bias_t[:], scale=SCALE)

                # idx = sum_k enc*k, split DVE | GpSimd+ACT
                junk_a = junkp.tile([128, SPLIT], f32, tag="ja")
                idx_a = smallp.tile([128, 1], f32, tag="ia")
                nc.vector.scalar_tensor_tensor(
                    junk_a[:], enc_t[:, :SPLIT], 0.0, iota[:, :SPLIT],
                    op0=Alu.add, op1=Alu.mult, accum_out=idx_a[:])
                junk_b = junkp.tile([128, K - SPLIT], f32, tag="jb")
                nc.gpsimd.tensor_tensor(junk_b[:], enc_t[:, SPLIT:], iota[:, SPLIT:], op=Alu.mult)
                junk_b2 = junkp.tile([128, K - SPLIT], f32, tag="jb2")
                idx_b = smallp.tile([128, 1], f32, tag="ib")
                nc.scalar.activation(junk_b2[:], junk_b[:], Act.Copy, accum_out=idx_b[:])
                if t % QG == 0:
                    idx_q = qidxp.tile([128, QG], mybir.dt.int32, tag="idxq")
                idx_f = smallp.tile([128, 1], f32, tag="if")
                nc.vector.tensor_add(idx_f[:], idx_a[:], idx_b[:])
                nc.vector.tensor_copy(idx_q[:, t % QG:t % QG + 1], idx_f[:])

                if not NO_ENC:
                    nc.sync.dma_start(enc_out[t * 128:(t + 1) * 128, :], enc_t[:])

                q_t = qp.tile([128, D], f32)
                if NO_Q:
                    nc.vector.memset(q_t[:], 0.0)
                else:
                    nc.gpsimd.indirect_dma_start(
                        out=q_t[:], out_offset=None, in_=e_nat[:],
                        in_offset=bass.IndirectOffsetOnAxis(ap=idx_q[:, :1], axis=0),
                        bounds_check=K - 1, oob_is_err=False)
                nc.sync.dma_start(q_out[t * 128:(t + 1) * 128, :], q_t[:])

            st = acc.tile([128, 1], f32)
            nc.vector.tensor_reduce(st[:, 0:1], m_all[:], axis=mybir.AxisListType.X, op=Alu.add)
            nc.sync.dma_start(stats[:], st[:])

    nc.compile()
    return nc


def _get_nc(n_tiles=NT):
    if n_tiles not in _CACHE:
        _CACHE[n_tiles] = _build_nc(n_tiles)
    return _CACHE[n_tiles]


def _split_bf16(a):
    import ml_dtypes
    hi = a.astype(ml_dtypes.bfloat16)
    lo = (a - hi.astype(np.float32)).astype(ml_dtypes.bfloat16)
    return hi, lo


def _host_inputs(c_input, embedding_weight):
    import ml_dtypes
    e = np.ascontiguousarray(embedding_weight, dtype=np.float32)
    et = e.T.astype(np.float32)
    eh, el = _split_bf16(et)
    esq = np.sum(e ** 2, axis=1).astype(np.float32)
    sh, sl = _split_bf16(-esq)
    blob = np.zeros((128, 3 * K + 128), ml_dtypes.bfloat16)
    blob[:, :K] = eh
    blob[:, K:2 * K] = el
    blob[0, 2 * K:3 * K] = sh
    blob[1, 2 * K:3 * K] = sl
    blob[0:2, 3 * K:3 * K + 128] = 1.0
    iotaf = np.tile(np.arange(K, dtype=np.float32)[None, :], (128, 1))

    in_maps = []
    for c in range(NCORES):
        shard = c_input[c * NSHARD:(c + 1) * NSHARD].astype(np.float32, copy=False)
        x2t = 2.0 * shard.T  # [D, ns]
        xh, xl = _split_bf16(x2t)
        ns_c = x2t.shape[1]
        hl = np.empty((D, 2 * ns_c), ml_dtypes.bfloat16)
        h3 = hl.reshape(D, ns_c // 128, 256)
        h3[:, :, :128] = xh.reshape(D, ns_c // 128, 128)
        h3[:, :, 128:] = xl.reshape(D, ns_c // 128, 128)
        in_maps.append({"xt_hl": hl, "blob": blob, "iotaf": iotaf, "e_nat": e})
    return in_maps


def _run(c_input, embedding_weight, **kw):
    from concourse.bass_utils import run_bass_kernel_spmd

    nc = _get_nc()
    in_maps = _host_inputs(np.asarray(c_input), np.asarray(embedding_weight))
    return run_bass_kernel_spmd(nc, in_maps, list(range(NCORES)), **kw)


def _assemble(results, xsq_total):
    enc = np.concatenate([r["enc_out"] for r in results], axis=0)
    q = np.concatenate([r["q_out"] for r in results], axis=0)
    sum_m = sum(r["stats"].astype(np.float64).sum() for r in results)
    loss = np.float32(1.25 * (xsq_total - sum_m) / (N * D))
    return loss, q, enc


def _xsq_total(c_input):
    x = np.asarray(c_input, dtype=np.float32)
    return float(np.sum(x.astype(np.float64) ** 2))


def kernel(c_input, embedding_weight):
    res = _run(c_input, embedding_weight)
    return _assemble(res.results, _xsq_total(c_input))


def kernel_profiled(c_input, embedding_weight):
    """Returns ((loss, q, enc), BassKernelResults) with trace enabled."""
    res = _run(c_input, embedding_weight, trace=True)
    return _assemble(res.results, _xsq_total(c_input)), res


# revision 20
# speedup vs baseline: 1.8166x; 1.0615x over previous
"""VQ codebook forward (nn_CodeBook) on 8 Trainium2 NeuronCores.

Math (per row i of c_input):
    dist[i,k] = |x_i|^2 + |e_k|^2 - 2 x_i.e_k
    argmin_k dist = argmax_k negdist,  negdist[i,k] = 2 x_i.e_k - |e_k|^2
    min-dist     = |x_i|^2 - m_i,      m_i = max_k negdist[i,k]
    loss = 1.25 * mean(min-dist)   (q/e latent losses are numerically equal)
    quantized_st == e[argmin]      (straight-through is identity in value)
    encodings = one_hot(argmin)

Device kernel per 128-row tile (rows on partitions, K=1024 on free dim):
    PE   : negdist2 = 2 x.e^T via matmul (x pre-transposed on host)
    DVE  : tensor_tensor_reduce fuses PSUM evacuation, -|e|^2 bias and
           row-max m in one op
    ACT  : one-hot = Exp(2^50*(negdist - m)) -- exact {0.0, 1.0} since the
           pow-2 scale keeps the cancellation sign-exact; Square+accum for
           |x|^2 partial sums
    DVE/GpSimd/ACT: idx = sum_k onehot*k split across engines
    SWDGE: q rows gathered from the codebook with per-partition indirect DMA
Loss is assembled on host from per-core partial sums of m and |x|^2.
"""
import sys

sys.path.insert(0, "/opt/trn_rl_repo")

import numpy as np

N, K, D = 131072, 1024, 128
NCORES = 8
NSHARD = N // NCORES          # 16384 rows per core
NT = NSHARD // 128            # 128 tiles per core
SCALE = float(2.0 ** 50)
SPLIT = 524                   # idx columns handled by DVE stt; rest GpSimd+ACT
QG = 1                        # per-tile q-gather: HW indirect DMA takes one offset per partition

_CACHE = {}
NO_Q = False     # debug: skip indirect q gather
NO_ENC = False   # debug: skip enc DMA out


def _build_nc(n_tiles):
    import concourse.bacc as bacc
    import concourse.mybir as mybir
    import concourse.tile as tile
    from concourse import bass

    ns = n_tiles * 128
    nc = bacc.Bacc("TRN2", target_bir_lowering=False, debug=False, num_devices=NCORES)

    # x pre-scaled by 2, split hi/lo in bf16 on host, interleaved per 128-col tile:
    # [:, 256t:256t+128] = hi tile t, [:, 256t+128:256t+256] = lo tile t
    xt_hl = nc.declare_dram_parameter("xt_hl", [D, 2 * ns], mybir.dt.bfloat16, isOutput=False)
    # bf16 blob: [:,0:K]=e.T hi ; [:,K:2K]=e.T lo ;
    # rows0/1 [2K:3K] = -|e|^2 hi / lo ; rows0/1 [3K:3K+128] = 1.0
    blob = nc.declare_dram_parameter("blob", [128, 3 * K + 128], mybir.dt.bfloat16, isOutput=False)
    iotaf = nc.declare_dram_parameter("iotaf", [128, K], mybir.dt.float32, isOutput=False)
    e_nat = nc.declare_dram_parameter("e_nat", [K, D], mybir.dt.float32, isOutput=False)

    enc_out = nc.declare_dram_parameter("enc_out", [ns, K], mybir.dt.float32, isOutput=True)
    q_out = nc.declare_dram_parameter("q_out", [ns, D], mybir.dt.float32, isOutput=True)
    stats = nc.declare_dram_parameter("stats", [128, 1], mybir.dt.float32, isOutput=True)

    f32 = mybir.dt.float32
    Alu = mybir.AluOpType
    Act = mybir.ActivationFunctionType

    with tile.TileContext(nc) as tc:
        with (
            tc.tile_pool(name="const", bufs=1) as const,
            tc.tile_pool(name="acc", bufs=1) as acc,
            tc.tile_pool(name="xin", bufs=6) as xin,
            tc.tile_pool(name="nd", bufs=3) as ndp,
            tc.tile_pool(name="enc", bufs=4) as encp,
            tc.tile_pool(name="junk", bufs=2) as junkp,
            tc.tile_pool(name="small", bufs=4) as smallp,
            tc.tile_pool(name="q", bufs=3) as qp,
            tc.tile_pool(name="qidx", bufs=4) as qidxp,
            tc.tile_pool(name="ps", bufs=4, space="PSUM") as psp,
        ):
            bf16 = mybir.dt.bfloat16
            blob_t = const.tile([128, 3 * K + 128], bf16)
            nc.sync.dma_start(blob_t[:], blob[:])
            iota_t = const.tile([128, K], f32)
            nc.sync.dma_start(iota_t[:], iotaf[:])
            et_hi = blob_t[:, 0:K]
            et_lo = blob_t[:, K:2 * K]
            seed2 = blob_t[0:2, 2 * K:3 * K]
            ones2 = blob_t[0:2, 3 * K:3 * K + 128]
            iota = iota_t[:, :]

            m_all = acc.tile([128, n_tiles], f32)

            idx_q = None
            GT = 4  # tiles per matmul-emission group (matches PSUM bufs)
            assert n_tiles % GT == 0
            SL = [slice(0, 512), slice(512, 1024)]
            for g in range(n_tiles // GT):
                tiles = list(range(g * GT, g * GT + GT))
                xts = {}
                for t in tiles:
                    xhl_t = xin.tile([D, 256], bf16, tag="xhl")
                    nc.sync.dma_start(xhl_t[:], xt_hl[:, t * 256:(t + 1) * 256])
                    xts[t] = xhl_t
                nds = {}
                # all seed matmuls first: single stationary operand (ones2)
                for t in tiles:
                    nd_ps = psp.tile([128, K], f32, tag="nd")
                    nds[t] = nd_ps
                    for sl in SL:
                        nc.tensor.matmul(nd_ps[:, sl], ones2, seed2[:, sl], start=True, stop=False)
                # main matmuls, grouped by stationary operand per tile
                for t in tiles:
                    nd_ps, xhl_t = nds[t], xts[t]
                    xh_t, xl_t = xhl_t[:, 0:128], xhl_t[:, 128:256]
                    for sl in SL:
                        nc.tensor.matmul(nd_ps[:, sl], xh_t, et_hi[:, sl], start=False, stop=False)
                    for sl in SL:
                        nc.tensor.matmul(nd_ps[:, sl], xh_t, et_lo[:, sl], start=False, stop=False)
                    for sl in SL:
                        nc.tensor.matmul(nd_ps[:, sl], xl_t, et_hi[:, sl], start=False, stop=True)
              
                for t in tiles:
                    nd_ps = nds[t]
                    nc.vector.tensor_reduce(m_all[:, t:t + 1], nd_ps[:],
                                            axis=mybir.AxisListType.X, op=Alu.max)

                    bias_t = smallp.tile([128, 1], f32)
                    nc.scalar.mul(bias_t[:], m_all[:, t:t + 1], -SCALE)

                    enc_t = encp.tile([128, K], f32)
                    nc.scalar.activation(enc_t[:], nd_ps[:], Act.Exp, bias=bias_t[:], scale=SCALE)

                    # idx = sum_k enc*k, split DVE | GpSimd+ACT
                    junk_a = junkp.tile([128, SPLIT], f32, tag="ja")
                    idx_a = smallp.tile([128, 1], f32, tag="ia")
                    nc.vector.scalar_tensor_tensor(
                        junk_a[:], enc_t[:, :SPLIT], 0.0, iota[:, :SPLIT],
                        op0=Alu.add, op1=Alu.mult, accum_out=idx_a[:])
                    junk_b = junkp.tile([128, K - SPLIT], f32, tag="jb")
                    nc.gpsimd.tensor_tensor(junk_b[:], enc_t[:, SPLIT:], iota[:, SPLIT:], op=Alu.mult)
                    junk_b2 = junkp.tile([128, K - SPLIT], f32, tag="jb2")
                    idx_b = smallp.tile([128, 1], f32, tag="ib")
                    nc.scalar.activation(junk_b2[:], junk_b[:], Act.Copy, accum_out=idx_b[:])
                    idx_q = qidxp.tile([128, 1], mybir.dt.int32, tag="idxq")
                    idx_f = smallp.tile([128, 1], f32, tag="if")
                    nc.vector.tensor_add(idx_f[:], idx_a[:], idx_b[:])
                    nc.vector.tensor_copy(idx_q[:, 0:1], idx_f[:])

                    if not NO_ENC:
                        nc.sync.dma_start(enc_out[t * 128:(t + 1) * 128, :], enc_t[:])

                    q_t = qp.tile([128, D], f32)
                    if NO_Q:
                        nc.vector.memset(q_t[:], 0.0)
                    else:
                        nc.gpsimd.indirect_dma_start(
                            out=q_t[:], out_offset=None, in_=e_nat[:],
                            in_offset=bass.IndirectOffsetOnAxis(ap=idx_q[:, :1], axis=0),
                            bounds_check=K - 1, oob_is_err=False)
                    nc.sync.dma_start(q_out[t * 128:(t + 1) * 128, :], q_t[:])

            st = acc.tile([128, 1], f32)
            nc.vector.tensor_reduce(st[:, 0:1], m_all[:], axis=mybir.AxisListType.X, op=Alu.add)
            nc.sync.dma_start(stats[:], st[:])

    nc.compile()
    return nc


def _get_nc(n_tiles=NT):
    if n_tiles not in _CACHE:
        _CACHE[n_tiles] = _build_nc(n_tiles)
    return _CACHE[n_tiles]


def _split_bf16(a):
    import ml_dtypes
    hi = a.astype(ml_dtypes.bfloat16)
    lo = (a - hi.astype(np.float32)).astype(ml_dtypes.bfloat16)
    return hi, lo


def _host_inputs(c_input, embedding_weight):
    import ml_dtypes
    e = np.ascontiguousarray(embedding_weight, dtype=np.float32)
    et = e.T.astype(np.float32)
    eh, el = _split_bf16(et)
    esq = np.sum(e ** 2, axis=1).astype(np.float32)
    sh, sl = _split_bf16(-esq)
    blob = np.zeros((128, 3 * K + 128), ml_dtypes.bfloat16)
    blob[:, :K] = eh
    blob[:, K:2 * K] = el
    blob[0, 2 * K:3 * K] = sh
    blob[1, 2 * K:3 * K] = sl
    blob[0:2, 3 * K:3 * K + 128] = 1.0
    iotaf = np.tile(np.arange(K, dtype=np.float32)[None, :], (128, 1))

    in_maps = []
    for c in range(NCORES):
        shard = c_input[c * NSHARD:(c + 1) * NSHARD].astype(np.float32, copy=False)
        x2t = 2.0 * shard.T  # [D, ns]
        xh, xl = _split_bf16(x2t)
        ns_c = x2t.shape[1]
        hl = np.empty((D, 2 * ns_c), ml_dtypes.bfloat16)
        h3 = hl.reshape(D, ns_c // 128, 256)
        h3[:, :, :128] = xh.reshape(D, ns_c // 128, 128)
        h3[:, :, 128:] = xl.reshape(D, ns_c // 128, 128)
        in_maps.append({"xt_hl": hl, "blob": blob, "iotaf": iotaf, "e_nat": e})
    return in_maps


def _run(c_input, embedding_weight, **kw):
    from concourse.bass_utils import run_bass_kernel_spmd

    nc = _get_nc()
    in_maps = _host_inputs(np.asarray(c_input), np.asarray(embedding_weight))
    return run_bass_kernel_spmd(nc, in_maps, list(range(NCORES)), **kw)


def _assemble(results, xsq_total):
    enc = np.concatenate([r["enc_out"] for r in results], axis=0)
    q = np.concatenate([r["q_out"] for r in results], axis=0)
    sum_m = sum(r["stats"].astype(np.float64).sum() for r in results)
    loss = np.float32(1.25 * (xsq_total - sum_m) / (N * D))
    return loss, q, enc


def _xsq_total(c_input):
    x = np.asarray(c_input, dtype=np.float32)
    return float(np.sum(x.astype(np.float64) ** 2))


def kernel(c_input, embedding_weight):
    res = _run(c_input, embedding_weight)
    return _assemble(res.results, _xsq_total(c_input))


def kernel_profiled(c_input, embedding_weight):
    """Returns ((loss, q, enc), BassKernelResults) with trace enabled."""
    res = _run(c_input, embedding_weight, trace=True)
    return _assemble(res.results, _xsq_total(c_input)), res


# revision 23
# speedup vs baseline: 2.2890x; 1.2600x over previous
"""VQ codebook forward (nn_CodeBook) on 8 Trainium2 NeuronCores.

Math (per row i of c_input):
    dist[i,k] = |x_i|^2 + |e_k|^2 - 2 x_i.e_k
    argmin_k dist = argmax_k negdist,  negdist[i,k] = 2 x_i.e_k - |e_k|^2
    min-dist     = |x_i|^2 - m_i,      m_i = max_k negdist[i,k]
    loss = 1.25 * mean(min-dist)   (q/e latent losses are numerically equal)
    quantized_st == e[argmin]      (straight-through is identity in value)
    encodings = one_hot(argmin)

Device kernel per 128-row tile (rows on partitions, K=1024 on free dim):
    PE   : negdist2 = 2 x.e^T via matmul (x pre-transposed on host)
    DVE  : tensor_tensor_reduce fuses PSUM evacuation, -|e|^2 bias and
           row-max m in one op
    ACT  : one-hot = Exp(2^50*(negdist - m)) -- exact {0.0, 1.0} since the
           pow-2 scale keeps the cancellation sign-exact; Square+accum for
           |x|^2 partial sums
    DVE/GpSimd/ACT: idx = sum_k onehot*k split across engines
    SWDGE: q rows gathered from the codebook with per-partition indirect DMA
Loss is assembled on host from per-core partial sums of m and |x|^2.
"""
import sys

sys.path.insert(0, "/opt/trn_rl_repo")

import numpy as np

N, K, D = 131072, 1024, 128
NCORES = 8
NSHARD = N // NCORES          # 16384 rows per core
NT = NSHARD // 128            # 128 tiles per core
SCALE = float(2.0 ** 50)
SPLIT = 524                   # idx columns handled by DVE stt; rest GpSimd+ACT
QG = 1                        # per-tile q-gather: HW indirect DMA takes one offset per partition

_CACHE = {}
NO_Q = False     # debug: skip indirect q gather
NO_ENC = False   # debug: skip enc DMA out


def _enable_ldw_opt():
    """Flip walrus --enable-ldw-opt: dedups back-to-back LDWEIGHTS with the
    same stationary operand (we emit runs of 8/4/2 same-lhsT matmuls)."""
    import concourse.bass_utils as bu
    if getattr(bu.run_command, "_ldw_patched", False):
        return
    orig = bu.run_command

    def patched(argv, **kw):
        argv = [a.replace("--enable-ldw-opt=false", "--enable-ldw-opt=true")
                if isinstance(a, str) else a for a in argv]
        return orig(argv, **kw)

    patched._ldw_patched = True
    bu.run_command = patched




def _build_nc(n_tiles):
    import concourse.bacc as bacc
    import concourse.mybir as mybir
    import concourse.tile as tile
    from concourse import bass

    ns = n_tiles * 128
    nc = bacc.Bacc("TRN2", target_bir_lowering=False, debug=False, num_devices=NCORES)

    # x pre-scaled by 2, split hi/lo in bf16 on host, interleaved per 128-col tile:
    # [:, 256t:256t+128] = hi tile t, [:, 256t+128:256t+256] = lo tile t
    xt_hl = nc.declare_dram_parameter("xt_hl", [D, 2 * ns], mybir.dt.bfloat16, isOutput=False)
    # bf16 blob: [:,0:K]=e.T hi ; [:,K:2K]=e.T lo ;
    # rows0/1 [2K:3K] = -|e|^2 hi / lo ; rows0/1 [3K:3K+128] = 1.0
    blob = nc.declare_dram_parameter("blob", [128, 3 * K + 128], mybir.dt.bfloat16, isOutput=False)
    iotaf = nc.declare_dram_parameter("iotaf", [128, K], mybir.dt.float32, isOutput=False)
    e_nat = nc.declare_dram_parameter("e_nat", [K, D], mybir.dt.float32, isOutput=False)

    enc_out = nc.declare_dram_parameter("enc_out", [ns, K], mybir.dt.float32, isOutput=True)
    q_out = nc.declare_dram_parameter("q_out", [ns, D], mybir.dt.float32, isOutput=True)
    stats = nc.declare_dram_parameter("stats", [128, 1], mybir.dt.float32, isOutput=True)

    f32 = mybir.dt.float32
    Alu = mybir.AluOpType
    Act = mybir.ActivationFunctionType

    with tile.TileContext(nc) as tc:
        with (
            tc.tile_pool(name="const", bufs=1) as const,
            tc.tile_pool(name="acc", bufs=1) as acc,
            tc.tile_pool(name="xin", bufs=6) as xin,
            tc.tile_pool(name="nd", bufs=3) as ndp,
            tc.tile_pool(name="enc", bufs=4) as encp,
            tc.tile_pool(name="junk", bufs=2) as junkp,
            tc.tile_pool(name="small", bufs=4) as smallp,
            tc.tile_pool(name="q", bufs=3) as qp,
            tc.tile_pool(name="qidx", bufs=4) as qidxp,
            tc.tile_pool(name="ps", bufs=4, space="PSUM") as psp,
        ):
            bf16 = mybir.dt.bfloat16
            blob_t = const.tile([128, 3 * K + 128], bf16)
            nc.sync.dma_start(blob_t[:], blob[:])
            iota_t = const.tile([128, K], f32)
            nc.sync.dma_start(iota_t[:], iotaf[:])
            et_hi = blob_t[:, 0:K]
            et_lo = blob_t[:, K:2 * K]
            seed2 = blob_t[0:2, 2 * K:3 * K]
            ones2 = blob_t[0:2, 3 * K:3 * K + 128]
            iota = iota_t[:, :]

            m_all = acc.tile([128, n_tiles], f32)

            idx_q = None
            GT = 2  # tiles per matmul group; psum bufs=4 lets 2 groups overlap (keeps PE warm)
            assert n_tiles % GT == 0
            SL = [slice(0, 512), slice(512, 1024)]
            for g in range(n_tiles // GT):
                tiles = list(range(g * GT, g * GT + GT))
                xts = {}
                for t in tiles:
                    xhl_t = xin.tile([D, 256], bf16, tag="xhl")
                    nc.sync.dma_start(xhl_t[:], xt_hl[:, t * 256:(t + 1) * 256])
                    xts[t] = xhl_t
                nds = {}
                # all seed matmuls first: single stationary operand (ones2)
                for t in tiles:
                    nd_ps = psp.tile([128, K], f32, tag="nd")
                    nds[t] = nd_ps
                    for sl in SL:
                        nc.tensor.matmul(nd_ps[:, sl], ones2, seed2[:, sl], start=True, stop=False)
                # main matmuls, grouped by stationary operand per tile
                for t in tiles:
                    nd_ps, xhl_t = nds[t], xts[t]
                    xh_t, xl_t = xhl_t[:, 0:128], xhl_t[:, 128:256]
                    for sl in SL:
                        nc.tensor.matmul(nd_ps[:, sl], xh_t, et_hi[:, sl], start=False, stop=False)
                    for sl in SL:
                        nc.tensor.matmul(nd_ps[:, sl], xh_t, et_lo[:, sl], start=False, stop=False)
                    for sl in SL:
                        nc.tensor.matmul(nd_ps[:, sl], xl_t, et_hi[:, sl], start=False, stop=True)
              
                for t in tiles:
                    nd_ps = nds[t]
                    nc.vector.tensor_reduce(m_all[:, t:t + 1], nd_ps[:],
                                            axis=mybir.AxisListType.X, op=Alu.max)

                    bias_t = smallp.tile([128, 1], f32)
                    nc.scalar.mul(bias_t[:], m_all[:, t:t + 1], -SCALE)

                    enc_t = encp.tile([128, K], f32)
                    nc.scalar.activation(enc_t[:], nd_ps[:], Act.Exp, bias=bias_t[:], scale=SCALE)

                    # idx = sum_k enc*k in one DVE pass; int cast on GpSimd
                    junk_a = junkp.tile([128, K], f32, tag="ja")
                    idx_f = smallp.tile([128, 1], f32, tag="if")
                    nc.vector.scalar_tensor_tensor(
                        junk_a[:], enc_t[:], 0.0, iota,
                        op0=Alu.add, op1=Alu.mult, accum_out=idx_f[:])
                    idx_q = qidxp.tile([128, 1], mybir.dt.int32, tag="idxq")
                    nc.gpsimd.tensor_copy(idx_q[:, 0:1], idx_f[:])

                    if not NO_ENC:
                        nc.sync.dma_start(enc_out[t * 128:(t + 1) * 128, :], enc_t[:])

                    q_t = qp.tile([128, D], f32)
                    if NO_Q:
                        nc.vector.memset(q_t[:], 0.0)
                    else:
                        nc.gpsimd.indirect_dma_start(
                            out=q_t[:], out_offset=None, in_=e_nat[:],
                            in_offset=bass.IndirectOffsetOnAxis(ap=idx_q[:, :1], axis=0),
                            bounds_check=K - 1, oob_is_err=False)
                    nc.scalar.dma_start(q_out[t * 128:(t + 1) * 128, :], q_t[:])

            st = acc.tile([128, 1], f32)
            nc.vector.tensor_reduce(st[:, 0:1], m_all[:], axis=mybir.AxisListType.X, op=Alu.add)
            nc.sync.dma_start(stats[:], st[:])

    nc.compile()
    return nc


def _get_nc(n_tiles=NT):
    if n_tiles not in _CACHE:
        _CACHE[n_tiles] = _build_nc(n_tiles)
    return _CACHE[n_tiles]


def _split_bf16(a):
    import ml_dtypes
    hi = a.astype(ml_dtypes.bfloat16)
    lo = (a - hi.astype(np.float32)).astype(ml_dtypes.bfloat16)
    return hi, lo


def _host_inputs(c_input, embedding_weight):
    import ml_dtypes
    e = np.ascontiguousarray(embedding_weight, dtype=np.float32)
    et = e.T.astype(np.float32)
    eh, el = _split_bf16(et)
    esq = np.sum(e ** 2, axis=1).astype(np.float32)
    sh, sl = _split_bf16(-esq)
    blob = np.zeros((128, 3 * K + 128), ml_dtypes.bfloat16)
    blob[:, :K] = eh
    blob[:, K:2 * K] = el
    blob[0, 2 * K:3 * K] = sh
    blob[1, 2 * K:3 * K] = sl
    blob[0:2, 3 * K:3 * K + 128] = 1.0
    iotaf = np.tile(np.arange(K, dtype=np.float32)[None, :], (128, 1))

    in_maps = []
    for c in range(NCORES):
        shard = c_input[c * NSHARD:(c + 1) * NSHARD].astype(np.float32, copy=False)
        x2t = 2.0 * shard.T  # [D, ns]
        xh, xl = _split_bf16(x2t)
        ns_c = x2t.shape[1]
        hl = np.empty((D, 2 * ns_c), ml_dtypes.bfloat16)
        h3 = hl.reshape(D, ns_c // 128, 256)
        h3[:, :, :128] = xh.reshape(D, ns_c // 128, 128)
        h3[:, :, 128:] = xl.reshape(D, ns_c // 128, 128)
        in_maps.append({"xt_hl": hl, "blob": blob, "iotaf": iotaf, "e_nat": e})
    return in_maps


def _run(c_input, embedding_weight, **kw):
    from concourse.bass_utils import run_bass_kernel_spmd

    nc = _get_nc()
    in_maps = _host_inputs(np.asarray(c_input), np.asarray(embedding_weight))
    return run_bass_kernel_spmd(nc, in_maps, list(range(NCORES)), **kw)


def _assemble(results, xsq_total):
    enc = np.concatenate([r["enc_out"] for r in results], axis=0)
    q = np.concatenate([r["q_out"] for r in results], axis=0)
    sum_m = sum(r["stats"].astype(np.float64).sum() for r in results)
    loss = np.float32(1.25 * (xsq_total - sum_m) / (N * D))
    return loss, q, enc


def _xsq_total(c_input):
    x = np.asarray(c_input, dtype=np.float32)
    return float(np.sum(x.astype(np.float64) ** 2))


def kernel(c_input, embedding_weight):
    res = _run(c_input, embedding_weight)
    return _assemble(res.results, _xsq_total(c_input))


def kernel_profiled(c_input, embedding_weight):
    """Returns ((loss, q, enc), BassKernelResults) with trace enabled."""
    res = _run(c_input, embedding_weight, trace=True)
    return _assemble(res.results, _xsq_total(c_input)), res


# revision 24
# speedup vs baseline: 2.5005x; 1.0924x over previous
"""VQ codebook forward (nn_CodeBook) on 8 Trainium2 NeuronCores.

Math (per row i of c_input):
    dist[i,k] = |x_i|^2 + |e_k|^2 - 2 x_i.e_k
    argmin_k dist = argmax_k negdist,  negdist[i,k] = 2 x_i.e_k - |e_k|^2
    min-dist     = |x_i|^2 - m_i,      m_i = max_k negdist[i,k]
    loss = 1.25 * mean(min-dist)   (q/e latent losses are numerically equal)
    quantized_st == e[argmin]      (straight-through is identity in value)
    encodings = one_hot(argmin)

Device kernel per 128-row tile (rows on partitions, K=1024 on free dim):
    PE   : negdist2 = 2 x.e^T via matmul (x pre-transposed on host)
    DVE  : tensor_tensor_reduce fuses PSUM evacuation, -|e|^2 bias and
           row-max m in one op
    ACT  : one-hot = Exp(2^50*(negdist - m)) -- exact {0.0, 1.0} since the
           pow-2 scale keeps the cancellation sign-exact; Square+accum for
           |x|^2 partial sums
    DVE/GpSimd/ACT: idx = sum_k onehot*k split across engines
    SWDGE: q rows gathered from the codebook with per-partition indirect DMA
Loss is assembled on host from per-core partial sums of m and |x|^2.
"""
import sys

sys.path.insert(0, "/opt/trn_rl_repo")

import numpy as np

N, K, D = 131072, 1024, 128
NCORES = 8
NSHARD = N // NCORES          # 16384 rows per core
NT = NSHARD // 128            # 128 tiles per core
SCALE = float(2.0 ** 50)
SPLIT = 524                   # idx columns handled by DVE stt; rest GpSimd+ACT
QG = 1                        # per-tile q-gather: HW indirect DMA takes one offset per partition

_CACHE = {}
NO_Q = False     # debug: skip indirect q gather
NO_ENC = False   # debug: skip enc DMA out


def _enable_ldw_opt():
    """Flip walrus --enable-ldw-opt: dedups back-to-back LDWEIGHTS with the
    same stationary operand (we emit runs of 8/4/2 same-lhsT matmuls)."""
    import concourse.bass_utils as bu
    if getattr(bu.run_command, "_ldw_patched", False):
        return
    orig = bu.run_command

    def patched(argv, **kw):
        argv = [a.replace("--enable-ldw-opt=false", "--enable-ldw-opt=true")
                if isinstance(a, str) else a for a in argv]
        return orig(argv, **kw)

    patched._ldw_patched = True
    bu.run_command = patched




def _build_nc(n_tiles):
    import concourse.bacc as bacc
    import concourse.mybir as mybir
    import concourse.tile as tile
    from concourse import bass

    ns = n_tiles * 128
    nc = bacc.Bacc("TRN2", target_bir_lowering=False, debug=False, num_devices=NCORES)

    # x pre-scaled by 2, split hi/lo in bf16 on host, interleaved per 128-col tile:
    # [:, 256t:256t+128] = hi tile t, [:, 256t+128:256t+256] = lo tile t
    xt_hl = nc.declare_dram_parameter("xt_hl", [D, 2 * ns], mybir.dt.bfloat16, isOutput=False)
    # bf16 blob: [:,0:K]=e.T hi ; [:,K:2K]=e.T lo ;
    # rows0/1 [2K:3K] = -|e|^2 hi / lo ; rows0/1 [3K:3K+128] = 1.0
    blob = nc.declare_dram_parameter("blob", [128, 3 * K + 128], mybir.dt.bfloat16, isOutput=False)
    iotaf = nc.declare_dram_parameter("iotaf", [128, K], mybir.dt.float32, isOutput=False)
    e_nat = nc.declare_dram_parameter("e_nat", [K, D], mybir.dt.float32, isOutput=False)

    enc_out = nc.declare_dram_parameter("enc_out", [ns, K], mybir.dt.float32, isOutput=True)
    q_out = nc.declare_dram_parameter("q_out", [ns, D], mybir.dt.float32, isOutput=True)
    stats = nc.declare_dram_parameter("stats", [128, 1], mybir.dt.float32, isOutput=True)

    f32 = mybir.dt.float32
    Alu = mybir.AluOpType
    Act = mybir.ActivationFunctionType

    with tile.TileContext(nc) as tc:
        with (
            tc.tile_pool(name="const", bufs=1) as const,
            tc.tile_pool(name="acc", bufs=1) as acc,
            tc.tile_pool(name="xin", bufs=6) as xin,
            tc.tile_pool(name="nd", bufs=3) as ndp,
            tc.tile_pool(name="enc", bufs=4) as encp,
            tc.tile_pool(name="junk", bufs=2) as junkp,
            tc.tile_pool(name="small", bufs=4) as smallp,
            tc.tile_pool(name="q", bufs=3) as qp,
            tc.tile_pool(name="qidx", bufs=4) as qidxp,
            tc.tile_pool(name="ps", bufs=4, space="PSUM") as psp,
        ):
            bf16 = mybir.dt.bfloat16
            blob_t = const.tile([128, 3 * K + 128], bf16)
            nc.sync.dma_start(blob_t[:], blob[:])
            iota_t = const.tile([128, K], f32)
            nc.sync.dma_start(iota_t[:], iotaf[:])
            et_hi = blob_t[:, 0:K]
            et_lo = blob_t[:, K:2 * K]
            seed2 = blob_t[0:2, 2 * K:3 * K]
            ones2 = blob_t[0:2, 3 * K:3 * K + 128]
            iota = iota_t[:, :]

            m_all = acc.tile([128, n_tiles], f32)

            idx_q = None
            GT = 2  # tiles per matmul group; psum bufs=4 lets 2 groups overlap (keeps PE warm)
            assert n_tiles % GT == 0
            SL = [slice(0, 512), slice(512, 1024)]
            for g in range(n_tiles // GT):
                tiles = list(range(g * GT, g * GT + GT))
                xts = {}
                for t in tiles:
                    xhl_t = xin.tile([D, 256], bf16, tag="xhl")
                    nc.sync.dma_start(xhl_t[:], xt_hl[:, t * 256:(t + 1) * 256])
                    xts[t] = xhl_t
                nds = {}
                # all seed matmuls first: single stationary operand (ones2)
                for t in tiles:
                    nd_ps = psp.tile([128, K], f32, tag="nd")
                    nds[t] = nd_ps
                    for sl in SL:
                        nc.tensor.matmul(nd_ps[:, sl], ones2, seed2[:, sl], start=True, stop=False)
                # main matmuls, grouped by stationary operand per tile
                for t in tiles:
                    nd_ps, xhl_t = nds[t], xts[t]
                    xh_t, xl_t = xhl_t[:, 0:128], xhl_t[:, 128:256]
                    for sl in SL:
                        nc.tensor.matmul(nd_ps[:, sl], xh_t, et_hi[:, sl], start=False, stop=False)
                    for sl in SL:
                        nc.tensor.matmul(nd_ps[:, sl], xh_t, et_lo[:, sl], start=False, stop=False)
                    for sl in SL:
                        nc.tensor.matmul(nd_ps[:, sl], xl_t, et_hi[:, sl], start=False, stop=True)
              
                for t in tiles:
                    nd_ps = nds[t]
                    nc.vector.tensor_reduce(m_all[:, t:t + 1], nd_ps[:],
                                            axis=mybir.AxisListType.X, op=Alu.max)

                    bias_t = smallp.tile([128, 1], f32)
                    nc.scalar.mul(bias_t[:], m_all[:, t:t + 1], -SCALE)

                    enc_t = encp.tile([128, K], f32)
                    nc.scalar.activation(enc_t[:], nd_ps[:], Act.Exp, bias=bias_t[:], scale=SCALE)

                    # idx = sum_k enc*k in one DVE pass; int cast on GpSimd
                    junk_a = junkp.tile([128, K], f32, tag="ja")
                    idx_f = smallp.tile([128, 1], f32, tag="if")
                    nc.vector.scalar_tensor_tensor(
                        junk_a[:], enc_t[:], 0.0, iota,
                        op0=Alu.add, op1=Alu.mult, accum_out=idx_f[:])
                    idx_q = qidxp.tile([128, 1], mybir.dt.int32, tag="idxq")
                    nc.gpsimd.tensor_copy(idx_q[:, 0:1], idx_f[:])

                    if not NO_ENC:
                        nc.scalar.dma_start(enc_out[t * 128:(t + 1) * 128, :], enc_t[:])

                    q_t = qp.tile([128, D], f32)
                    if NO_Q:
                        nc.vector.memset(q_t[:], 0.0)
                    else:
                        nc.gpsimd.indirect_dma_start(
                            out=q_t[:], out_offset=None, in_=e_nat[:],
                            in_offset=bass.IndirectOffsetOnAxis(ap=idx_q[:, :1], axis=0),
                            bounds_check=K - 1, oob_is_err=False)
                    nc.sync.dma_start(q_out[t * 128:(t + 1) * 128, :], q_t[:])

            st = acc.tile([128, 1], f32)
            nc.vector.tensor_reduce(st[:, 0:1], m_all[:], axis=mybir.AxisListType.X, op=Alu.add)
            nc.sync.dma_start(stats[:], st[:])

    nc.compile()
    return nc


def _get_nc(n_tiles=NT):
    if n_tiles not in _CACHE:
        _CACHE[n_tiles] = _build_nc(n_tiles)
    return _CACHE[n_tiles]


def _split_bf16(a):
    import ml_dtypes
    hi = a.astype(ml_dtypes.bfloat16)
    lo = (a - hi.astype(np.float32)).astype(ml_dtypes.bfloat16)
    return hi, lo


def _host_inputs(c_input, embedding_weight):
    import ml_dtypes
    e = np.ascontiguousarray(embedding_weight, dtype=np.float32)
    et = e.T.astype(np.float32)
    eh, el = _split_bf16(et)
    esq = np.sum(e ** 2, axis=1).astype(np.float32)
    sh, sl = _split_bf16(-esq)
    blob = np.zeros((128, 3 * K + 128), ml_dtypes.bfloat16)
    blob[:, :K] = eh
    blob[:, K:2 * K] = el
    blob[0, 2 * K:3 * K] = sh
    blob[1, 2 * K:3 * K] = sl
    blob[0:2, 3 * K:3 * K + 128] = 1.0
    iotaf = np.tile(np.arange(K, dtype=np.float32)[None, :], (128, 1))

    in_maps = []
    for c in range(NCORES):
        shard = c_input[c * NSHARD:(c + 1) * NSHARD].astype(np.float32, copy=False)
        x2t = 2.0 * shard.T  # [D, ns]
        xh, xl = _split_bf16(x2t)
        ns_c = x2t.shape[1]
        hl = np.empty((D, 2 * ns_c), ml_dtypes.bfloat16)
        h3 = hl.reshape(D, ns_c // 128, 256)
        h3[:, :, :128] = xh.reshape(D, ns_c // 128, 128)
        h3[:, :, 128:] = xl.reshape(D, ns_c // 128, 128)
        in_maps.append({"xt_hl": hl, "blob": blob, "iotaf": iotaf, "e_nat": e})
    return in_maps


def _run(c_input, embedding_weight, **kw):
    from concourse.bass_utils import run_bass_kernel_spmd

    nc = _get_nc()
    in_maps = _host_inputs(np.asarray(c_input), np.asarray(embedding_weight))
    return run_bass_kernel_spmd(nc, in_maps, list(range(NCORES)), **kw)


def _assemble(results, xsq_total):
    enc = np.concatenate([r["enc_out"] for r in results], axis=0)
    q = np.concatenate([r["q_out"] for r in results], axis=0)
    sum_m = sum(r["stats"].astype(np.float64).sum() for r in results)
    loss = np.float32(1.25 * (xsq_total - sum_m) / (N * D))
    return loss, q, enc


def _xsq_total(c_input):
    x = np.asarray(c_input, dtype=np.float32)
    return float(np.sum(x.astype(np.float64) ** 2))


def kernel(c_input, embedding_weight):
    res = _run(c_input, embedding_weight)
    return _assemble(res.results, _xsq_total(c_input))


def kernel_profiled(c_input, embedding_weight):
    """Returns ((loss, q, enc), BassKernelResults) with trace enabled."""
    res = _run(c_input, embedding_weight, trace=True)
    return _assemble(res.results, _xsq_total(c_input)), res


# revision 25
# speedup vs baseline: 2.8119x; 1.1245x over previous
"""VQ codebook forward (nn_CodeBook) on 8 Trainium2 NeuronCores.

Math (per row i of c_input):
    dist[i,k] = |x_i|^2 + |e_k|^2 - 2 x_i.e_k
    argmin_k dist = argmax_k negdist,  negdist[i,k] = 2 x_i.e_k - |e_k|^2
    min-dist     = |x_i|^2 - m_i,      m_i = max_k negdist[i,k]
    loss = 1.25 * mean(min-dist)   (q/e latent losses are numerically equal)
    quantized_st == e[argmin]      (straight-through is identity in value)
    encodings = one_hot(argmin)

Device kernel per 128-row tile (rows on partitions, K=1024 on free dim):
    PE   : negdist2 = 2 x.e^T via matmul (x pre-transposed on host)
    DVE  : tensor_tensor_reduce fuses PSUM evacuation, -|e|^2 bias and
           row-max m in one op
    ACT  : one-hot = Exp(2^50*(negdist - m)) -- exact {0.0, 1.0} since the
           pow-2 scale keeps the cancellation sign-exact; Square+accum for
           |x|^2 partial sums
    DVE/GpSimd/ACT: idx = sum_k onehot*k split across engines
    SWDGE: q rows gathered from the codebook with per-partition indirect DMA
Loss is assembled on host from per-core partial sums of m and |x|^2.
"""
import sys

sys.path.insert(0, "/opt/trn_rl_repo")

import numpy as np

N, K, D = 131072, 1024, 128
NCORES = 8
NSHARD = N // NCORES          # 16384 rows per core
NT = NSHARD // 128            # 128 tiles per core
SCALE = float(2.0 ** 50)
SPLIT = 524                   # idx columns handled by DVE stt; rest GpSimd+ACT
QG = 1                        # per-tile q-gather: HW indirect DMA takes one offset per partition

_CACHE = {}
NO_Q = False     # debug: skip indirect q gather
NO_ENC = False   # debug: skip enc DMA out


def _enable_ldw_opt():
    """Flip walrus --enable-ldw-opt: dedups back-to-back LDWEIGHTS with the
    same stationary operand (we emit runs of 8/4/2 same-lhsT matmuls)."""
    import concourse.bass_utils as bu
    if getattr(bu.run_command, "_ldw_patched", False):
        return
    orig = bu.run_command

    def patched(argv, **kw):
        argv = [a.replace("--enable-ldw-opt=false", "--enable-ldw-opt=true")
                if isinstance(a, str) else a for a in argv]
        return orig(argv, **kw)

    patched._ldw_patched = True
    bu.run_command = patched




def _build_nc(n_tiles):
    import concourse.bacc as bacc
    import concourse.mybir as mybir
    import concourse.tile as tile
    from concourse import bass

    ns = n_tiles * 128
    nc = bacc.Bacc("TRN2", target_bir_lowering=False, debug=False, num_devices=NCORES)

    # x pre-scaled by 2, split hi/lo in bf16 on host, interleaved per 128-col tile:
    # [:, 256t:256t+128] = hi tile t, [:, 256t+128:256t+256] = lo tile t
    xt_hl = nc.declare_dram_parameter("xt_hl", [D, 2 * ns], mybir.dt.bfloat16, isOutput=False)
    # bf16 blob: [:,0:K]=e.T hi ; [:,K:2K]=e.T lo ;
    # rows0/1 [2K:3K] = -|e|^2 hi / lo ; rows0/1 [3K:3K+128] = 1.0
    blob = nc.declare_dram_parameter("blob", [128, 3 * K + 128], mybir.dt.bfloat16, isOutput=False)
    iotaf = nc.declare_dram_parameter("iotaf", [128, K], mybir.dt.float16, isOutput=False)
    e_nat = nc.declare_dram_parameter("e_nat", [K, D], mybir.dt.float32, isOutput=False)

    enc_out = nc.declare_dram_parameter("enc_out", [ns, K], mybir.dt.float16, isOutput=True)
    q_out = nc.declare_dram_parameter("q_out", [ns, D], mybir.dt.float32, isOutput=True)
    stats = nc.declare_dram_parameter("stats", [128, 1], mybir.dt.float32, isOutput=True)

    f32 = mybir.dt.float32
    Alu = mybir.AluOpType
    Act = mybir.ActivationFunctionType

    with tile.TileContext(nc) as tc:
        with (
            tc.tile_pool(name="const", bufs=1) as const,
            tc.tile_pool(name="acc", bufs=1) as acc,
            tc.tile_pool(name="xin", bufs=6) as xin,
            tc.tile_pool(name="nd", bufs=3) as ndp,
            tc.tile_pool(name="enc", bufs=4) as encp,
            tc.tile_pool(name="junk", bufs=2) as junkp,
            tc.tile_pool(name="small", bufs=4) as smallp,
            tc.tile_pool(name="q", bufs=3) as qp,
            tc.tile_pool(name="qidx", bufs=4) as qidxp,
            tc.tile_pool(name="ps", bufs=4, space="PSUM") as psp,
        ):
            bf16 = mybir.dt.bfloat16
            blob_t = const.tile([128, 3 * K + 128], bf16)
            nc.sync.dma_start(blob_t[:], blob[:])
            iota_t = const.tile([128, K], mybir.dt.float16)
            nc.sync.dma_start(iota_t[:], iotaf[:])
            et_hi = blob_t[:, 0:K]
            et_lo = blob_t[:, K:2 * K]
            seed2 = blob_t[0:2, 2 * K:3 * K]
            ones2 = blob_t[0:2, 3 * K:3 * K + 128]
            iota = iota_t[:, :]

            m_all = acc.tile([128, n_tiles], f32)

            idx_q = None
            GT = 2  # tiles per matmul group; psum bufs=4 lets 2 groups overlap (keeps PE warm)
            assert n_tiles % GT == 0
            SL = [slice(0, 512), slice(512, 1024)]
            for g in range(n_tiles // GT):
                tiles = list(range(g * GT, g * GT + GT))
                xts = {}
                for t in tiles:
                    xhl_t = xin.tile([D, 256], bf16, tag="xhl")
                    nc.sync.dma_start(xhl_t[:], xt_hl[:, t * 256:(t + 1) * 256])
                    xts[t] = xhl_t
                nds = {}
                # all seed matmuls first: single stationary operand (ones2)
                for t in tiles:
                    nd_ps = psp.tile([128, K], f32, tag="nd")
                    nds[t] = nd_ps
                    for sl in SL:
                        nc.tensor.matmul(nd_ps[:, sl], ones2, seed2[:, sl], start=True, stop=False)
                # main matmuls, grouped by stationary operand per tile
                for t in tiles:
                    nd_ps, xhl_t = nds[t], xts[t]
                    xh_t, xl_t = xhl_t[:, 0:128], xhl_t[:, 128:256]
                    for sl in SL:
                        nc.tensor.matmul(nd_ps[:, sl], xh_t, et_hi[:, sl], start=False, stop=False)
                    for sl in SL:
                        nc.tensor.matmul(nd_ps[:, sl], xh_t, et_lo[:, sl], start=False, stop=False)
                    for sl in SL:
                        nc.tensor.matmul(nd_ps[:, sl], xl_t, et_hi[:, sl], start=False, stop=True)
              
                for t in tiles:
                    nd_ps = nds[t]
                    nc.vector.tensor_reduce(m_all[:, t:t + 1], nd_ps[:],
                                            axis=mybir.AxisListType.X, op=Alu.max)

                    bias_t = smallp.tile([128, 1], f32)
                    nc.scalar.mul(bias_t[:], m_all[:, t:t + 1], -SCALE)

                    enc_t = encp.tile([128, K], mybir.dt.float16)
                    nc.scalar.activation(enc_t[:], nd_ps[:], Act.Exp, bias=bias_t[:], scale=SCALE)

                    # idx = sum_k enc*k in one DVE pass; int cast on GpSimd
                    junk_a = junkp.tile([128, K], mybir.dt.float16, tag="ja")
                    idx_f = smallp.tile([128, 1], f32, tag="if")
                    nc.vector.scalar_tensor_tensor(
                        junk_a[:], enc_t[:], 0.0, iota,
                        op0=Alu.add, op1=Alu.mult, accum_out=idx_f[:])
                    idx_q = qidxp.tile([128, 1], mybir.dt.int32, tag="idxq")
                    nc.gpsimd.tensor_copy(idx_q[:, 0:1], idx_f[:])

                    if not NO_ENC:
                        nc.scalar.dma_start(enc_out[t * 128:(t + 1) * 128, :], enc_t[:])

                    q_t = qp.tile([128, D], f32)
                    if NO_Q:
                        nc.vector.memset(q_t[:], 0.0)
                    else:
                        nc.gpsimd.indirect_dma_start(
                            out=q_t[:], out_offset=None, in_=e_nat[:],
                            in_offset=bass.IndirectOffsetOnAxis(ap=idx_q[:, :1], axis=0),
                            bounds_check=K - 1, oob_is_err=False)
                    nc.sync.dma_start(q_out[t * 128:(t + 1) * 128, :], q_t[:])

            st = acc.tile([128, 1], f32)
            nc.vector.tensor_reduce(st[:, 0:1], m_all[:], axis=mybir.AxisListType.X, op=Alu.add)
            nc.sync.dma_start(stats[:], st[:])

    nc.compile()
    return nc


def _get_nc(n_tiles=NT):
    if n_tiles not in _CACHE:
        _CACHE[n_tiles] = _build_nc(n_tiles)
    return _CACHE[n_tiles]


def _split_bf16(a):
    import ml_dtypes
    hi = a.astype(ml_dtypes.bfloat16)
    lo = (a - hi.astype(np.float32)).astype(ml_dtypes.bfloat16)
    return hi, lo


def _host_inputs(c_input, embedding_weight):
    import ml_dtypes
    e = np.ascontiguousarray(embedding_weight, dtype=np.float32)
    et = e.T.astype(np.float32)
    eh, el = _split_bf16(et)
    esq = np.sum(e ** 2, axis=1).astype(np.float32)
    sh, sl = _split_bf16(-esq)
    blob = np.zeros((128, 3 * K + 128), ml_dtypes.bfloat16)
    blob[:, :K] = eh
    blob[:, K:2 * K] = el
    blob[0, 2 * K:3 * K] = sh
    blob[1, 2 * K:3 * K] = sl
    blob[0:2, 3 * K:3 * K + 128] = 1.0
    iotaf = np.tile(np.arange(K, dtype=np.float16)[None, :], (128, 1))

    in_maps = []
    for c in range(NCORES):
        shard = c_input[c * NSHARD:(c + 1) * NSHARD].astype(np.float32, copy=False)
        x2t = 2.0 * shard.T  # [D, ns]
        xh, xl = _split_bf16(x2t)
        ns_c = x2t.shape[1]
        hl = np.empty((D, 2 * ns_c), ml_dtypes.bfloat16)
        h3 = hl.reshape(D, ns_c // 128, 256)
        h3[:, :, :128] = xh.reshape(D, ns_c // 128, 128)
        h3[:, :, 128:] = xl.reshape(D, ns_c // 128, 128)
        in_maps.append({"xt_hl": hl, "blob": blob, "iotaf": iotaf, "e_nat": e})
    return in_maps


def _run(c_input, embedding_weight, **kw):
    from concourse.bass_utils import run_bass_kernel_spmd

    nc = _get_nc()
    in_maps = _host_inputs(np.asarray(c_input), np.asarray(embedding_weight))
    return run_bass_kernel_spmd(nc, in_maps, list(range(NCORES)), **kw)


def _assemble(results, xsq_total):
    enc = np.concatenate([r["enc_out"].astype(np.float32) for r in results], axis=0)
    q = np.concatenate([r["q_out"] for r in results], axis=0)
    sum_m = sum(r["stats"].astype(np.float64).sum() for r in results)
    loss = np.float32(1.25 * (xsq_total - sum_m) / (N * D))
    return loss, q, enc


def _xsq_total(c_input):
    x = np.asarray(c_input, dtype=np.float32)
    return float(np.sum(x.astype(np.float64) ** 2))


def kernel(c_input, embedding_weight):
    res = _run(c_input, embedding_weight)
    return _assemble(res.results, _xsq_total(c_input))


def kernel_profiled(c_input, embedding_weight):
    """Returns ((loss, q, enc), BassKernelResults) with trace enabled."""
    res = _run(c_input, embedding_weight, trace=True)
    return _assemble(res.results, _xsq_total(c_input)), res
